# revision 17
# baseline (speedup 1.0000x reference)
"""Trainium2 Bass kernel for causal multi-head attention with RoPE.

Problem: B=2, S=2048, D=1024, H=16 heads (Dh=64), causal, fp32, scores
scaled by 1/sqrt(D).

Sharding: data-parallel over batch (2) x tensor-parallel over heads (4)
on 8 NeuronCores. Each core projects 4 heads (qw/kw/vw column-sharded,
ow row-sharded), runs attention for those heads, computes a partial
output projection, then a ReduceScatter(+add) over its 4-core group
yields a [512, 1024] slice of the final output per core. The host
concatenates slices.

Device algorithm per core (TileContext):
  - Projections QT/KT [d, q] and V [kv, d] via fp32r (FP22) matmuls from
    host-pretransposed xT; RoPE applied in the transposed layout using a
    permutation matmul (rot) + cos/sin elementwise combines. The RoPE
    pair permutation and the 1/sqrt(D) scale are folded into qw/kw on
    the host.
  - Scores S[q, kv] per head in fp32r, causal block-sparse; row max via
    DVE reduce; exp via ACT with bias=-max and accumulated row sum;
    softmax weights written as fp16.
  - Weights transposed [128x128] via DMA-transpose; attention output
    OT[d, q] accumulated via fp16 matmuls (two heads column-packed);
    1/rowsum applied on OT (per-head row vector staged via PE transpose).
  - Output projection in fp32r; stores go straight PSUM->DRAM; then
    ReduceScatter over the 4-core group.
"""

import sys

sys.path.insert(0, "/opt/trn_rl_repo")

import math

import numpy as np

import concourse.bass as bass
import concourse.mybir as mybir
import concourse.tile as tile

f32 = mybir.dt.float32
f32r = mybir.dt.float32r
f16 = mybir.dt.float16
AF = mybir.ActivationFunctionType
AX = mybir.AxisListType
ALU = mybir.AluOpType

P = 128
S = 2048
D = 1024
DH = 64
HPC = 4  # heads per core
DCOL = HPC * DH  # 256: per-core projection width
NT = S // P  # 16 q tiles
NG = 4  # q groups of 512
GW = 512
CH = 1024  # score chunk width (2 PSUM banks)
NCK = D // P  # 8 contraction chunks
NEG = -1.0e10
N_CORES = 8


def _split_multi_waits(nc, limit=1):
    """Walrus codegen rejects instructions with >limit sem waits; offload
    extra waits onto NoOps inserted right before the instruction."""
    n_split = 0
    for _bbname, bb in nc.bb_map.items():
        insts = bb.bb.instructions
        new_list = []
        for inst in insts:
            si = inst.sync_info
            if si is not None and si.on_wait and len(si.on_wait) > limit:
                waits = list(si.on_wait)
                for w in waits[:-limit]:
                    nop = mybir.InstNoOp(
                        name=nc.get_next_instruction_name(), ins=[], outs=[]
                    )
                    nop.engine = inst.engine
                    nop.sync_info = mybir.SyncInfo(on_wait=[w], on_update=[])
                    new_list.append(nop)
                    n_split += 1
                si.on_wait = waits[-limit:]
            new_list.append(inst)
        insts[:] = new_list
    return n_split


def _emit(nc, tc):
    xT_d = nc.dram_tensor("xT", [D, S], f32, kind="ExternalInput").ap()
    qw_d = nc.dram_tensor("qw", [D, DCOL], f32, kind="ExternalInput").ap()
    kw_d = nc.dram_tensor("kw", [D, DCOL], f32, kind="ExternalInput").ap()
    vw_d = nc.dram_tensor("vw", [D, DCOL], f32, kind="ExternalInput").ap()
    ow_d = nc.dram_tensor("ow", [DCOL, D], f32, kind="ExternalInput").ap()
    cos_d = nc.dram_tensor("cosd", [P, S], f32, kind="ExternalInput").ap()
    sin_d = nc.dram_tensor("sind", [P, S], f32, kind="ExternalInput").ap()
    rot_d = nc.dram_tensor("rotm", [P, P], f32, kind="ExternalInput").ap()
    msk_d = nc.dram_tensor("maska", [P, P], f32, kind="ExternalInput").ap()
    y_d = nc.dram_tensor("y", [S // 4, D], f32, kind="ExternalOutput").ap()
    dbg = getattr(_emit, "debug", False)
    if dbg:
        dqt_d = nc.dram_tensor("dbg_qt0", [P, S], f32, kind="ExternalOutput").ap()
        dkt_d = nc.dram_tensor("dbg_kt0", [P, S], f32, kind="ExternalOutput").ap()
        dv_d = nc.dram_tensor("dbg_v", [P, NT, DCOL], f32, kind="ExternalOutput").ap()
        dot_d = nc.dram_tensor("dbg_ot0", [P, S], f32, kind="ExternalOutput").ap()
        dyl_d = nc.dram_tensor("dbg_yloc", [S, D], f32, kind="ExternalOutput").ap()
        dwa_d = nc.dram_tensor("dbg_wa", [P, CH], f32, kind="ExternalOutput").ap()
        dwt_d = nc.dram_tensor("dbg_wt", [P, 4, P], f32, kind="ExternalOutput").ap()

    ctx = _emit.ctx
    const = ctx.enter_context(tc.tile_pool(name="const", bufs=1))
    xts_p = ctx.enter_context(tc.tile_pool(name="xts", bufs=2))
    qkt_p = ctx.enter_context(tc.tile_pool(name="qkt", bufs=1))
    scr_p = ctx.enter_context(tc.tile_pool(name="scr", bufs=2))
    wa_p = ctx.enter_context(tc.tile_pool(name="wa", bufs=3))
    wt_p = ctx.enter_context(tc.tile_pool(name="wt", bufs=26))
    ot_p = ctx.enter_context(tc.tile_pool(name="ot", bufs=1))
    st_p = ctx.enter_context(tc.tile_pool(name="st", bufs=6))
    pss = ctx.enter_context(tc.tile_pool(name="pss", bufs=3, space="PSUM"))
    pm = ctx.enter_context(tc.tile_pool(name="pm", bufs=2, space="PSUM"))
    dram = ctx.enter_context(tc.tile_pool(name="dram", bufs=2, space="DRAM"))

    # ---- constant loads ----
    qw_s = const.tile([P, NCK, DCOL], f32r)
    nc.sync.dma_start(qw_s[:], qw_d.bitcast(f32r).rearrange("(c p) d -> p c d", p=P))
    kw_s = const.tile([P, NCK, DCOL], f32r)
    nc.sync.dma_start(kw_s[:], kw_d.bitcast(f32r).rearrange("(c p) d -> p c d", p=P))
    vw_s = const.tile([P, NCK, DCOL], f32r)
    nc.sync.dma_start(vw_s[:], vw_d.bitcast(f32r).rearrange("(c p) d -> p c d", p=P))
    ow_s = const.tile([P, 2, D], f32r)
    nc.sync.dma_start(ow_s[:], ow_d.bitcast(f32r).rearrange("(o p) n -> p o n", p=P))
    cos_s = const.tile([P, S], f32)
    nc.sync.dma_start(cos_s[:], cos_d[:])
    sin_s = const.tile([P, S], f32)
    nc.sync.dma_start(sin_s[:], sin_d[:])
    rot_s = const.tile([P, P], f32r)
    nc.sync.dma_start(rot_s[:], rot_d.bitcast(f32r)[:])
    msk_s = const.tile([P, P], f32)
    nc.sync.dma_start(msk_s[:], msk_d[:])

    v_s = const.tile([P, NT, DCOL], f16)  # V[kv, d], all 4 heads
    qt = [qkt_p.tile([P, S], f32r, name=f"qt{i}", tag=f"qt{i}") for i in range(2)]  # roped QT per pair
    kt = [qkt_p.tile([P, S], f32r, name=f"kt{i}", tag=f"kt{i}") for i in range(2)]  # roped KT per pair
    ot = [ot_p.tile([P, S], f32r, name=f"ot{i}", tag=f"ot{i}") for i in range(2)]  # OT per pair
    y_loc = dram.tile([S, D], f32)
    y_red = dram.tile([S // 4, D], f32)

    xT_r = xT_d.bitcast(f32r).rearrange("(c p) q -> p c q", p=P)

    def rope_project(pair, g, w_s, dst):
        """dst[:, 512g:+512] = rope(x @ w)^T for head pair `pair`."""
        gs = slice(g * GW, (g + 1) * GW)
        ps = pm.tile([P, GW], f32, tag="pm", name="pqk")
        for ck in range(NCK):
            nc.tensor.matmul(
                ps[:],
                w_s[:, ck, pair * P : (pair + 1) * P],
                xts[:, ck, :],
                start=(ck == 0),
                stop=(ck == NCK - 1),
            )
        raw = scr_p.tile([P, GW], f32r, tag="raw")
        nc.scalar.activation(raw[:], ps[:], AF.Copy)
        rp = pm.tile([P, GW], f32, tag="pm", name="prot")
        nc.tensor.matmul(rp[:], rot_s[:], raw[:], start=True, stop=True)
        t2 = scr_p.tile([P, GW], f32, tag="t2")
        nc.vector.tensor_tensor(t2[:], rp[:], sin_s[:, gs], ALU.mult)
        t1 = scr_p.tile([P, GW], f32, tag="t1")
        nc.gpsimd.tensor_tensor(t1[:], raw.bitcast(f32)[:], cos_s[:, gs], ALU.mult)
        nc.vector.tensor_tensor(dst[:, gs], t1[:], t2[:], ALU.add)

    # ---- phase 1: projections ----
    for g in range(NG):
        gs = slice(g * GW, (g + 1) * GW)
        xts = xts_p.tile([P, NCK, GW], f32r, tag="xts")
        nc.sync.dma_start(xts[:], xT_r[:, :, gs])
        for pair in range(2):
            rope_project(pair, g, qw_s, qt[pair])
            rope_project(pair, g, kw_s, kt[pair])
        for c4 in range(4):
            j = 4 * g + c4
            psv = pm.tile([P, DCOL], f32, tag="pm", name="psv")
            for ck in range(NCK):
                nc.tensor.matmul(
                    psv[:],
                    xts[:, ck, c4 * P : (c4 + 1) * P],
                    vw_s[:, ck, :],
                    start=(ck == 0),
                    stop=(ck == NCK - 1),
                )
            nc.scalar.activation(v_s[:, j, :], psv[:], AF.Copy)

    if dbg:
        nc.sync.dma_start(dqt_d[:], qt[0].bitcast(f32)[:])
        nc.sync.dma_start(dkt_d[:], kt[0].bitcast(f32)[:])
        nc.gpsimd.dma_start(dv_d[:], v_s[:])

    # ---- phase 2+3: scores/softmax/AV per pair ----
    for pair in range(2):
        wt_tiles = {}
        for g in range(NG):
            po = pm.tile([P, GW], f32, tag="pm", name="po")
            for h01 in range(2):
                base = DH * h01
                for t in range(4 * g, 4 * g + 4):
                    kvlen = P * (t + 1)
                    nch = (kvlen + CH - 1) // CH
                    lhsT_q = qt[pair][base : base + DH, t * P : (t + 1) * P]
                    mst = st_p.tile([P, 2], f32, tag="mst")
                    chunks = []
                    for ch in range(nch):
                        c0 = CH * ch
                        clen = min(CH, kvlen - c0)
                        ss = pss.tile([P, CH], f32, tag="ss")
                        for sub in range(0, clen, GW):
                            n = min(GW, clen - sub)
                            nc.tensor.matmul(
                                ss[:, sub : sub + n],
                                lhsT_q,
                                kt[pair][base : base + DH, c0 + sub : c0 + sub + n],
                                start=True,
                                stop=True,
                            )
                        chunks.append((ss, c0, clen))
                        if ch == nch - 1:
                            # diagonal causal mask on the last 128 columns
                            nc.vector.tensor_tensor(
                                ss[:, clen - P : clen],
                                ss[:, clen - P : clen],
                                msk_s[:],
                                ALU.add,
                            )
                        if nch == 1:
                            mneg = st_p.tile([P, 1], f32, tag="mneg")
                            nc.vector.tensor_reduce(
                                mneg[:], ss[:, :clen], axis=AX.X, op=ALU.max,
                                negate=True,
                            )
                        else:
                            nc.vector.tensor_reduce(
                                mst[:, ch : ch + 1], ss[:, :clen], axis=AX.X,
                                op=ALU.max,
                            )
                    if nch > 1:
                        mneg = st_p.tile([P, 1], f32, tag="mneg")
                        nc.vector.tensor_reduce(
                            mneg[:], mst[:, :nch], axis=AX.X, op=ALU.max, negate=True
                        )
                    sst = st_p.tile([P, 2], f32, tag="sst")
                    was = []
                    for ch, (ss, c0, clen) in enumerate(chunks):
                        wa = wa_p.tile([P, CH], f16, tag="wa")
                        nc.scalar.activation(
                            wa[:, :clen],
                            ss[:, :clen],
                            AF.Exp,
                            bias=mneg[:],
                            accum_out=sst[:, ch : ch + 1],
                        )
                        was.append((wa, c0, clen))
                    if nch > 1:
                        ssum = st_p.tile([P, 1], f32, tag="ssum")
                        nc.vector.tensor_reduce(
                            ssum[:], sst[:, :nch], axis=AX.X, op=ALU.add
                        )
                    else:
                        ssum = sst[:, 0:1]
                    rrec = st_p.tile([P, 1], f32, tag="rrec")
                    nc.vector.reciprocal(rrec[:], ssum[:])
                    # normalize then transpose W chunks into WT tiles
                    for wa, c0, clen in was:
                        nc.vector.tensor_scalar_mul(wa[:, :clen], wa[:, :clen], rrec[:])
                        for b in range(clen // P):
                            j = (c0 // P) + b
                            key = (h01, g, j)
                            if key not in wt_tiles:
                                wt_tiles[key] = wt_p.tile([P, 4, P], f16, name=f"wt_{key}", tag="wt")
                            nc.scalar.dma_start(
                                wt_tiles[key][:, t - 4 * g, :],
                                wa[:, b * P : (b + 1) * P],
                                transpose=True,
                            )
                    if dbg and pair == 0 and h01 == 0 and t == getattr(_emit, 'dbg_t', 8):
                        nc.gpsimd.dma_start(dwa_d[:], was[0][0][:])
                # AV for this head over the whole group (column-packed in po)
                hh = 2 * pair + h01
                ob = 64 * h01
                njs = 4 * g + 4
                for j in range(njs):
                    tmin = max(0, j - 4 * g)
                    nc.tensor.matmul(
                        po[ob : ob + 64, tmin * P : GW],
                        v_s[:, j, hh * DH : (hh + 1) * DH],
                        wt_tiles[(h01, g, j)][:, tmin:4, :],
                        start=(j == 0),
                        stop=(j == njs - 1),
                        skip_group_check=True,
                    )
                if dbg and pair == 0 and h01 == 0 and g == getattr(_emit, 'dbg_g', 2):
                    nc.gpsimd.dma_start(dwt_d[:], wt_tiles[(0, g, getattr(_emit, 'dbg_j', 3))][:])
                # release this head's WT tiles
                for j in range(njs):
                    del wt_tiles[(h01, g, j)]
            gs = slice(g * GW, (g + 1) * GW)
            nc.scalar.activation(ot[pair][:, gs], po[:], AF.Copy)

    if dbg:
        nc.sync.dma_start(dot_d[:], ot[0].bitcast(f32)[:])

    # ---- phase 4: output projection + reduce-scatter ----
    for t in range(NT):
        for nh in range(2):
            ns = slice(nh * GW, (nh + 1) * GW)
            py = pm.tile([P, GW], f32, tag="pm", name="py")
            nc.tensor.matmul(
                py[:], ot[0][:, t * P : (t + 1) * P], ow_s[:, 0, ns],
                start=True, stop=False,
            )
            nc.tensor.matmul(
                py[:], ot[1][:, t * P : (t + 1) * P], ow_s[:, 1, ns],
                start=False, stop=True,
            )
            ysb = scr_p.tile([P, GW], f32, tag="ysb", name="ysb")
            nc.scalar.activation(ysb[:], py[:], AF.Copy)
            nc.sync.dma_start(y_loc[t * P : (t + 1) * P, ns], ysb[:])

    if dbg:
        nc.sync.dma_start(dyl_d[:], y_loc[:])
    nc.gpsimd.collective_compute(
        "ReduceScatter",
        ALU.add,
        replica_groups=[[0, 1, 2, 3], [4, 5, 6, 7]],
        ins=[y_loc.opt()],
        outs=[y_red.opt()],
    )
    nc.sync.dma_start(y_d[:], y_red[:])


_BUILt = None


def _build():
    global _BUILt
    if _BUILt is not None:
        return _BUILt
    from contextlib import ExitStack

    nc = bass.Bass("TRN2", target_bir_lowering=False, debug=False, num_devices=N_CORES)
    with tile.TileContext(nc) as tc:
        with ExitStack() as ctx:
            _emit.ctx = ctx
            _emit(nc, tc)
    _split_multi_waits(nc)
    _BUILt = nc
    return nc


def _host_inputs(x, qw, kw, vw, ow):
    """Build the 8 per-core input dicts."""
    scale = 1.0 / math.sqrt(D)
    deint = np.concatenate([np.arange(0, DH, 2), np.arange(1, DH, 2)])
    qw_s = (qw.astype(np.float64) * scale).astype(np.float32)

    # RoPE tables in the transposed [d, q] layout (freq index = p mod 32)
    inv_freq = 1.0 / (10000.0 ** (np.arange(0, DH, 2, dtype=np.float64) / DH))
    pos = np.arange(S, dtype=np.float64)
    ang = np.outer(inv_freq, pos)  # [32, S]
    cosd = np.tile(np.cos(ang), (4, 1)).astype(np.float32)  # [128, S]
    sind = np.tile(np.sin(ang), (4, 1)).astype(np.float32)

    rotm = np.zeros((P, P), dtype=np.float32)
    for m in range(P):
        mm = m % DH
        if mm < DH // 2:
            rotm[m + DH // 2, m] = -1.0
        else:
            rotm[m - DH // 2, m] = 1.0

    maska = np.where(
        np.arange(P)[None, :] <= np.arange(P)[:, None], 0.0, NEG
    ).astype(np.float32)

    ins = []
    for c in range(N_CORES):
        b, r = divmod(c, 4)
        heads = [4 * r + h for h in range(HPC)]
        pcols = np.concatenate([DH * h + deint for h in heads])
        cols = np.concatenate([DH * h + np.arange(DH) for h in heads])
        ins.append(
            {
                "xT": np.ascontiguousarray(x[b].T),
                "qw": np.ascontiguousarray(qw_s[:, pcols]),
                "kw": np.ascontiguousarray(kw[:, pcols]),
                "vw": np.ascontiguousarray(vw[:, cols]),
                "ow": np.ascontiguousarray(ow[cols, :]),
                "cosd": cosd,
                "sind": sind,
                "rotm": rotm,
                "maska": maska,
            }
        )
    return ins


def run(x, qw, kw, vw, ow, trace=False, trace_cores=None):
    from concourse import bass_utils

    nc = _build()
    ins = _host_inputs(x, qw, kw, vw, ow)
    res = bass_utils.run_bass_kernel_spmd(
        nc,
        ins,
        core_ids=list(range(N_CORES)),
        trace=trace,
        trace_cores=trace_cores,
    )
    parts = [r["y"] for r in res.results]
    out = np.stack(
        [np.concatenate(parts[0:4], axis=0), np.concatenate(parts[4:8], axis=0)]
    )
    return out, res


def kernel(x, qw, kw, vw, ow):
    out, _ = run(
        np.asarray(x, dtype=np.float32),
        np.asarray(qw, dtype=np.float32),
        np.asarray(kw, dtype=np.float32),
        np.asarray(vw, dtype=np.float32),
        np.asarray(ow, dtype=np.float32),
    )
    return out


# revision 19
# speedup vs baseline: 13.7081x; 13.7081x over previous
"""Trainium2 Bass kernel for causal multi-head attention with RoPE.

Problem: B=2, S=2048, D=1024, H=16 heads (Dh=64), causal, fp32, scores
scaled by 1/sqrt(D).

Sharding: data-parallel over batch (2) x tensor-parallel over heads (4)
on 8 NeuronCores. Each core projects 4 heads (qw/kw/vw column-sharded,
ow row-sharded), runs attention for those heads, computes a partial
output projection, then a ReduceScatter(+add) over its 4-core group
yields a [512, 1024] slice of the final output per core. The host
concatenates slices.

Device algorithm per core (TileContext):
  - Projections QT/KT [d, q] and V [kv, d] via fp32r (FP22) matmuls from
    host-pretransposed xT; RoPE applied in the transposed layout using a
    permutation matmul (rot) + cos/sin elementwise combines. The RoPE
    pair permutation and the 1/sqrt(D) scale are folded into qw/kw on
    the host.
  - Scores S[q, kv] per head in fp32r, causal block-sparse; row max via
    DVE reduce; exp via ACT with bias=-max and accumulated row sum;
    softmax weights normalized by 1/rowsum (per-partition) and written
    as fp16.
  - Weights transposed [128x128] via DMA-transpose on the ACT HWDGE
    queues (kept disjoint from copy-mode DMAs on the SP queues - mode
    transitions within a queue corrupt data); attention output OT[d, q]
    accumulated via fp16 matmuls, two heads column-packed.
  - Output projection in fp32r; ReduceScatter over the 4-core group.
"""

import sys

sys.path.insert(0, "/opt/trn_rl_repo")

import math

import numpy as np

import concourse.bass as bass
import concourse.mybir as mybir
import concourse.tile as tile

f32 = mybir.dt.float32
f32r = mybir.dt.float32r
f16 = mybir.dt.float16
AF = mybir.ActivationFunctionType
AX = mybir.AxisListType
ALU = mybir.AluOpType

P = 128
S = 2048
D = 1024
DH = 64
HPC = 4  # heads per core
DCOL = HPC * DH  # 256: per-core projection width
NT = S // P  # 16 q tiles
NG = 4  # q groups of 512
GW = 512
CH = 1024  # score chunk width (2 PSUM banks)
NCK = D // P  # 8 contraction chunks
NEG = -1.0e10
N_CORES = 8


def _split_multi_waits(nc, limit=1):
    """Walrus codegen rejects instructions with >limit sem waits; offload
    extra waits onto NoOps inserted right before the instruction."""
    n_split = 0
    for _bbname, bb in nc.bb_map.items():
        insts = bb.bb.instructions
        new_list = []
        for inst in insts:
            si = inst.sync_info
            if si is not None and si.on_wait and len(si.on_wait) > limit:
                waits = list(si.on_wait)
                for w in waits[:-limit]:
                    nop = mybir.InstNoOp(
                        name=nc.get_next_instruction_name(), ins=[], outs=[]
                    )
                    nop.engine = inst.engine
                    nop.sync_info = mybir.SyncInfo(on_wait=[w], on_update=[])
                    new_list.append(nop)
                    n_split += 1
                si.on_wait = waits[-limit:]
            new_list.append(inst)
        insts[:] = new_list
    return n_split


def _emit(nc, tc, ctx, reps=1):
    xT_d = nc.dram_tensor("xT", [D, S], f32, kind="ExternalInput").ap()
    qw_d = nc.dram_tensor("qw", [D, DCOL], f32, kind="ExternalInput").ap()
    kw_d = nc.dram_tensor("kw", [D, DCOL], f32, kind="ExternalInput").ap()
    vw_d = nc.dram_tensor("vw", [D, DCOL], f32, kind="ExternalInput").ap()
    ow_d = nc.dram_tensor("ow", [DCOL, D], f32, kind="ExternalInput").ap()
    cos_d = nc.dram_tensor("cosd", [P, S], f32, kind="ExternalInput").ap()
    sin_d = nc.dram_tensor("sind", [P, S], f32, kind="ExternalInput").ap()
    rot_d = nc.dram_tensor("rotm", [P, P], f32, kind="ExternalInput").ap()
    msk_d = nc.dram_tensor("maska", [P, P], f32, kind="ExternalInput").ap()
    y_d = nc.dram_tensor("y", [S // 4, D], f32, kind="ExternalOutput").ap()

    const = ctx.enter_context(tc.tile_pool(name="const", bufs=1))
    xts_p = ctx.enter_context(tc.tile_pool(name="xts", bufs=2))
    qkt_p = ctx.enter_context(tc.tile_pool(name="qkt", bufs=1))
    scr_p = ctx.enter_context(tc.tile_pool(name="scr", bufs=2))
    wa_p = ctx.enter_context(tc.tile_pool(name="wa", bufs=3))
    wt_p = ctx.enter_context(tc.tile_pool(name="wt", bufs=26))
    ot_p = ctx.enter_context(tc.tile_pool(name="ot", bufs=1))
    st_p = ctx.enter_context(tc.tile_pool(name="st", bufs=6))
    pss = ctx.enter_context(tc.tile_pool(name="pss", bufs=3, space="PSUM"))
    pm = ctx.enter_context(tc.tile_pool(name="pm", bufs=2, space="PSUM"))
    dram = ctx.enter_context(tc.tile_pool(name="dram", bufs=2, space="DRAM"))

    # ---- constant loads ----
    qw_s = const.tile([P, NCK, DCOL], f32r)
    nc.sync.dma_start(qw_s[:], qw_d.bitcast(f32r).rearrange("(c p) d -> p c d", p=P))
    kw_s = const.tile([P, NCK, DCOL], f32r)
    nc.sync.dma_start(kw_s[:], kw_d.bitcast(f32r).rearrange("(c p) d -> p c d", p=P))
    vw_s = const.tile([P, NCK, DCOL], f32r)
    nc.sync.dma_start(vw_s[:], vw_d.bitcast(f32r).rearrange("(c p) d -> p c d", p=P))
    ow_s = const.tile([P, 2, D], f32r)
    nc.sync.dma_start(ow_s[:], ow_d.bitcast(f32r).rearrange("(o p) n -> p o n", p=P))
    cos_s = const.tile([P, S], f32)
    nc.sync.dma_start(cos_s[:], cos_d[:])
    sin_s = const.tile([P, S], f32)
    nc.sync.dma_start(sin_s[:], sin_d[:])
    rot_s = const.tile([P, P], f32r)
    nc.sync.dma_start(rot_s[:], rot_d.bitcast(f32r)[:])
    msk_s = const.tile([P, P], f32)
    nc.sync.dma_start(msk_s[:], msk_d[:])

    xT_r = xT_d.bitcast(f32r).rearrange("(c p) q -> p c q", p=P)

    def run_pass():
        v_s = qkt_p.tile([P, NT, DCOL], f16, tag="vs", name="vs")
        qt = [qkt_p.tile([P, S], f32r, name=f"qt{i}", tag=f"qt{i}") for i in range(2)]
        kt = [qkt_p.tile([P, S], f32r, name=f"kt{i}", tag=f"kt{i}") for i in range(2)]
        ot = [ot_p.tile([P, S], f32r, name=f"ot{i}", tag=f"ot{i}") for i in range(2)]
        y_loc = dram.tile([S, D], f32, tag="yloc", name="yloc")
        y_red = dram.tile([S // 4, D], f32, tag="yred", name="yred")
        box = {}

        def rope_project(pair, g, w_s, dst):
            """dst[:, 512g:+512] = rope(x @ w)^T for head pair `pair`."""
            gs = slice(g * GW, (g + 1) * GW)
            xts = box["xts"]
            ps = pm.tile([P, GW], f32, tag="pm", name="pqk")
            for ck in range(NCK):
                nc.tensor.matmul(
                    ps[:],
                    w_s[:, ck, pair * P : (pair + 1) * P],
                    xts[:, ck, :],
                    start=(ck == 0),
                    stop=(ck == NCK - 1),
                )
            raw = scr_p.tile([P, GW], f32r, tag="raw", name="raw")
            nc.scalar.activation(raw[:], ps[:], AF.Copy)
            rp = pm.tile([P, GW], f32, tag="pm", name="prot")
            nc.tensor.matmul(rp[:], rot_s[:], raw[:], start=True, stop=True)
            t2 = scr_p.tile([P, GW], f32, tag="t2", name="t2")
            nc.vector.tensor_tensor(t2[:], rp[:], sin_s[:, gs], ALU.mult)
            t1 = scr_p.tile([P, GW], f32, tag="t1", name="t1")
            nc.gpsimd.tensor_tensor(t1[:], raw.bitcast(f32)[:], cos_s[:, gs], ALU.mult)
            nc.vector.tensor_tensor(dst[:, gs], t1[:], t2[:], ALU.add)

        # ---- phase 1: projections ----
        for g in range(NG):
            gs = slice(g * GW, (g + 1) * GW)
            xts = xts_p.tile([P, NCK, GW], f32r, tag="xts", name="xts")
            box["xts"] = xts
            nc.sync.dma_start(xts[:], xT_r[:, :, gs])
            for pair in range(2):
                rope_project(pair, g, qw_s, qt[pair])
                rope_project(pair, g, kw_s, kt[pair])
            for c4 in range(4):
                j = 4 * g + c4
                psv = pm.tile([P, DCOL], f32, tag="pm", name="psv")
                for ck in range(NCK):
                    nc.tensor.matmul(
                        psv[:],
                        xts[:, ck, c4 * P : (c4 + 1) * P],
                        vw_s[:, ck, :],
                        start=(ck == 0),
                        stop=(ck == NCK - 1),
                    )
                nc.scalar.activation(v_s[:, j, :], psv[:], AF.Copy)

        # ---- phase 2+3: scores/softmax/AV per pair ----
        for pair in range(2):
            wt_tiles = {}
            for g in range(NG):
                po = pm.tile([P, GW], f32, tag="pm", name="po")
                for h01 in range(2):
                    base = DH * h01
                    for t in range(4 * g, 4 * g + 4):
                        kvlen = P * (t + 1)
                        nch = (kvlen + CH - 1) // CH
                        lhsT_q = qt[pair][base : base + DH, t * P : (t + 1) * P]
                        mst = st_p.tile([P, 2], f32, tag="mst", name="mst")
                        chunks = []
                        for ch in range(nch):
                            c0 = CH * ch
                            clen = min(CH, kvlen - c0)
                            ss = pss.tile([P, CH], f32, tag="ss", name="ss")
                            for sub in range(0, clen, GW):
                                n = min(GW, clen - sub)
                                nc.tensor.matmul(
                                    ss[:, sub : sub + n],
                                    lhsT_q,
                                    kt[pair][
                                        base : base + DH, c0 + sub : c0 + sub + n
                                    ],
                                    start=True,
                                    stop=True,
                                )
                            chunks.append((ss, c0, clen))
                            if ch == nch - 1:
                                # diagonal causal mask on the last 128 columns
                                nc.vector.tensor_tensor(
                                    ss[:, clen - P : clen],
                                    ss[:, clen - P : clen],
                                    msk_s[:],
                                    ALU.add,
                                )
                            if nch == 1:
                                mneg = st_p.tile([P, 1], f32, tag="mneg", name="mneg")
                                nc.vector.tensor_reduce(
                                    mneg[:], ss[:, :clen], axis=AX.X, op=ALU.max,
                                    negate=True,
                                )
                            else:
                                nc.vector.tensor_reduce(
                                    mst[:, ch : ch + 1], ss[:, :clen], axis=AX.X,
                                    op=ALU.max,
                                )
                        if nch > 1:
                            mneg = st_p.tile([P, 1], f32, tag="mneg", name="mneg")
                            nc.vector.tensor_reduce(
                                mneg[:], mst[:, :nch], axis=AX.X, op=ALU.max,
                                negate=True,
                            )
                        sst = st_p.tile([P, 2], f32, tag="sst", name="sst")
                        was = []
                        for ch, (ss, c0, clen) in enumerate(chunks):
                            wa = wa_p.tile([P, CH], f16, tag="wa", name="wa")
                            nc.scalar.activation(
                                wa[:, :clen],
                                ss[:, :clen],
                                AF.Exp,
                                bias=mneg[:],
                                accum_out=sst[:, ch : ch + 1],
                            )
                            was.append((wa, c0, clen))
                        if nch > 1:
                            ssum = st_p.tile([P, 1], f32, tag="ssum", name="ssum")
                            nc.vector.tensor_reduce(
                                ssum[:], sst[:, :nch], axis=AX.X, op=ALU.add
                            )
                        else:
                            ssum = sst[:, 0:1]
                        rrec = st_p.tile([P, 1], f32, tag="rrec", name="rrec")
                        nc.vector.reciprocal(rrec[:], ssum[:])
                        # normalize then transpose W chunks into WT tiles
                        for wa, c0, clen in was:
                            nc.vector.tensor_scalar_mul(
                                wa[:, :clen], wa[:, :clen], rrec[:]
                            )
                            for b in range(clen // P):
                                j = (c0 // P) + b
                                key = (h01, g, j)
                                if key not in wt_tiles:
                                    wt_tiles[key] = wt_p.tile(
                                        [P, 4, P], f16, name=f"wt{h01}_{g}_{j}",
                                        tag="wt",
                                    )
                                # ACT-engine queues: keep transpose-mode DMAs off
                                # the copy-mode (SP) queues
                                nc.scalar.dma_start(
                                    wt_tiles[key][:, t - 4 * g, :],
                                    wa[:, b * P : (b + 1) * P],
                                    transpose=True,
                                )
                    # AV for this head over the whole group (col-packed in po)
                    hh = 2 * pair + h01
                    ob = 64 * h01
                    njs = 4 * g + 4
                    for j in range(njs):
                        tmin = max(0, j - 4 * g)
                        nc.tensor.matmul(
                            po[ob : ob + 64, tmin * P : GW],
                            v_s[:, j, hh * DH : (hh + 1) * DH],
                            wt_tiles[(h01, g, j)][:, tmin:4, :],
                            start=(j == 0),
                            stop=(j == njs - 1),
                            skip_group_check=True,
                        )
                    for j in range(njs):
                        del wt_tiles[(h01, g, j)]
                gs = slice(g * GW, (g + 1) * GW)
                nc.scalar.activation(ot[pair][:, gs], po[:], AF.Copy)

        # ---- phase 4: output projection + reduce-scatter ----
        for t in range(NT):
            for nh in range(2):
                ns = slice(nh * GW, (nh + 1) * GW)
                py = pm.tile([P, GW], f32, tag="pm", name="py")
                nc.tensor.matmul(
                    py[:], ot[0][:, t * P : (t + 1) * P], ow_s[:, 0, ns],
                    start=True, stop=False,
                )
                nc.tensor.matmul(
                    py[:], ot[1][:, t * P : (t + 1) * P], ow_s[:, 1, ns],
                    start=False, stop=True,
                )
                ysb = scr_p.tile([P, GW], f32, tag="ysb", name="ysb")
                nc.scalar.activation(ysb[:], py[:], AF.Copy)
                nc.sync.dma_start(y_loc[t * P : (t + 1) * P, ns], ysb[:])

        nc.gpsimd.collective_compute(
            "ReduceScatter",
            ALU.add,
            replica_groups=[[0, 1, 2, 3], [4, 5, 6, 7]],
            ins=[y_loc.opt()],
            outs=[y_red.opt()],
        )
        nc.sync.dma_start(y_d[:], y_red[:])

    for _rep in range(reps):
        run_pass()


_BUILT = {}


def _build(reps=1):
    if reps in _BUILT:
        return _BUILT[reps]
    from contextlib import ExitStack

    nc = bass.Bass("TRN2", target_bir_lowering=False, debug=False, num_devices=N_CORES)
    with tile.TileContext(nc) as tc:
        with ExitStack() as ctx:
            _emit(nc, tc, ctx, reps=reps)
    _split_multi_waits(nc)
    _BUILT[reps] = nc
    return nc


def _host_inputs(x, qw, kw, vw, ow):
    """Build the 8 per-core input dicts."""
    scale = 1.0 / math.sqrt(D)
    deint = np.concatenate([np.arange(0, DH, 2), np.arange(1, DH, 2)])
    qw_s = (qw.astype(np.float64) * scale).astype(np.float32)

    # RoPE tables in the transposed [d, q] layout (freq index = p mod 32)
    inv_freq = 1.0 / (10000.0 ** (np.arange(0, DH, 2, dtype=np.float64) / DH))
    pos = np.arange(S, dtype=np.float64)
    ang = np.outer(inv_freq, pos)  # [32, S]
    cosd = np.tile(np.cos(ang), (4, 1)).astype(np.float32)  # [128, S]
    sind = np.tile(np.sin(ang), (4, 1)).astype(np.float32)

    rotm = np.zeros((P, P), dtype=np.float32)
    for m in range(P):
        mm = m % DH
        if mm < DH // 2:
            rotm[m + DH // 2, m] = -1.0
        else:
            rotm[m - DH // 2, m] = 1.0

    maska = np.where(
        np.arange(P)[None, :] <= np.arange(P)[:, None], 0.0, NEG
    ).astype(np.float32)

    ins = []
    for c in range(N_CORES):
        b, r = divmod(c, 4)
        heads = [4 * r + h for h in range(HPC)]
        pcols = np.concatenate([DH * h + deint for h in heads])
        cols = np.concatenate([DH * h + np.arange(DH) for h in heads])
        ins.append(
            {
                "xT": np.ascontiguousarray(x[b].T),
                "qw": np.ascontiguousarray(qw_s[:, pcols]),
                "kw": np.ascontiguousarray(kw[:, pcols]),
                "vw": np.ascontiguousarray(vw[:, cols]),
                "ow": np.ascontiguousarray(ow[cols, :]),
                "cosd": cosd,
                "sind": sind,
                "rotm": rotm,
                "maska": maska,
            }
        )
    return ins


def run(x, qw, kw, vw, ow, trace=False, trace_cores=None, reps=1):
    from concourse import bass_utils

    nc = _build(reps=reps)
    ins = _host_inputs(x, qw, kw, vw, ow)
    res = bass_utils.run_bass_kernel_spmd(
        nc,
        ins,
        core_ids=list(range(N_CORES)),
        trace=trace,
        trace_cores=trace_cores,
    )
    parts = [r["y"] for r in res.results]
    out = np.stack(
        [np.concatenate(parts[0:4], axis=0), np.concatenate(parts[4:8], axis=0)]
    )
    return out, res


def kernel(x, qw, kw, vw, ow):
    out, _ = run(
        np.asarray(x, dtype=np.float32),
        np.asarray(qw, dtype=np.float32),
        np.asarray(kw, dtype=np.float32),
        np.asarray(vw, dtype=np.float32),
        np.asarray(ow, dtype=np.float32),
    )
    return out
